# revision 48
# baseline (speedup 1.0000x reference)
"""Fused multi-head attention (B=4, S=2048, D=1024, H=16) on 8 trn2 cores.

Sharding: core = (batch b, head-half). Tensor parallel over heads: each core
projects its batch onto its 8 heads (512 of the 1024 q/k/v dims), runs full
attention over all 2048 queries x 2048 keys for those heads, and emits the
PARTIAL output projection y_part = o_heads @ Wo[:, slice].T as f32. The
all-reduce over the core pair (+ bias) happens host-side in the gather step.

vs. the batch x query-half sharding this halves the K/V projection work per
core (no duplication across the pair), dropping PE work from ~382us to
~273us serial-equivalent (~362us with the per-matmul LDWEIGHTS bubble this
toolchain can't hide), close to the scalar-engine exp floor (~286us).

Blocks run column-major over head pairs -- (sq0..3, h0), (sq0..3, h1), ... --
so K(hp) projections are only needed before column hp, Q(hp, sq) before block
(sq, hp), and the partial output projection weaves into the last column.

Layouts (feature dim on partitions, no transposes anywhere):
  qT[o,sq]   = wqT.T @ xqT        (bf16, evac + bias -> qT sbuf)
  ktT[o,sk]  = wkT.T @ xkT        (bf16, cached in SBUF)
  v[sk,o]    = xvT.T @ wvT        (bf16 per head + ones column)
  scoresT[sk,sq] = kt_h.T @ qT_h  (K=64; even/odd heads row-packed -> 2x)
  p = exp(scoresT/8)              (ACT, one exp per 2 PSUM banks, bf16)
  [oT_h; den] = [v_h|1].T @ p     (bf16, fp32 accum; sk_t-level pipelining)
  oT (unnormalized) + den rows evacuate to SBUF; per sq block a batched DVE
  recip + gpsimd bcast normalizes oT in place, deferred off the PE FIFO's
  critical path (O-proj matmul emission is lagged so it never heads the
  in-order PE queue before its normalization deps are done).
  yT[j,sq] = woT.T @ oT           (partial: no bias; host adds pair + byT)
"""

from collections import deque

import numpy as np

import concourse.bacc as bacc
import concourse.mybir as mybir
import concourse.tile as tile
from concourse.bass_utils import run_bass_kernel_spmd


B, S, D, H = 4, 2048, 1024, 16
DK = D // H          # 64
DH = D // 2          # 512 head dims per core
HLOC = H // 2        # 8 heads per core
NCORES = 8
NSK = S // 128       # 16 sk tiles
NOT = DH // 128      # 4 feature tiles (q/k/v dims of this core)
NIT = D // 128       # 8 contraction tiles for qkv projections
NSQ = S // 512       # 4 query blocks
NJT = D // 128       # 8 output feature tiles (full d_model)

f32 = mybir.dt.float32
bf16 = mybir.dt.bfloat16
f8 = mybir.dt.float8e4

_COMPILED = None


def build():
    nc = bacc.Bacc("TRN2", target_bir_lowering=False, debug=False)

    xqT = nc.dram_tensor("xqT", [D, S], bf16, kind="ExternalInput")
    xkT = nc.dram_tensor("xkT", [D, S], bf16, kind="ExternalInput")
    xvT = nc.dram_tensor("xvT", [D, S], bf16, kind="ExternalInput")
    wqT = nc.dram_tensor("wqT", [D, DH], bf16, kind="ExternalInput")
    wkT = nc.dram_tensor("wkT", [D, DH], bf16, kind="ExternalInput")
    wvT = nc.dram_tensor("wvT", [D, DH], bf16, kind="ExternalInput")
    woT = nc.dram_tensor("woT", [DH, D], bf16, kind="ExternalInput")
    bq = nc.dram_tensor("bq", [DH], f32, kind="ExternalInput")
    bk = nc.dram_tensor("bk", [DH], f32, kind="ExternalInput")
    yT = nc.dram_tensor("yT", [D, S], f32, kind="ExternalOutput")

    xqr = xqT.rearrange("(t p) m -> p t m", p=128)
    xkr = xkT.rearrange("(t p) m -> p t m", p=128)
    xvr = xvT.rearrange("(t p) m -> p t m", p=128)
    wqr = wqT.rearrange("(t p) m -> p t m", p=128)
    wkr = wkT.rearrange("(t p) m -> p t m", p=128)
    wvr = wvT.rearrange("(t p) m -> p t m", p=128)
    wor = woT.rearrange("(t p) m -> p t m", p=128)

    EXP = mybir.ActivationFunctionType.Exp

    with tile.TileContext(nc) as tc:
        with (
            tc.tile_pool(name="persist", bufs=1) as persist,
            tc.tile_pool(name="ps_s", bufs=2, space="PSUM") as ps_s,
            tc.tile_pool(name="ps_f", bufs=2, space="PSUM") as ps_f,
            tc.tile_pool(name="psv", bufs=1, space="PSUM") as psv,
            tc.tile_pool(name="ppool", bufs=11) as ppool,
            tc.tile_pool(name="xqp", bufs=2) as xqp,
            tc.tile_pool(name="xvp", bufs=4) as xvp,
            tc.tile_pool(name="ystg", bufs=2) as ystg_p,
            tc.tile_pool(name="recp", bufs=2) as recp,
            tc.tile_pool(name="bcp", bufs=1) as bcp,
        ):
            # ---- persistent tiles ----
            qT = persist.tile([128, NOT, S], bf16)            # 8KB/part
            kt = persist.tile([128, NOT, S], bf16)            # 8KB/part
            v_st = persist.tile([128, NSK, HLOC, DK + 1], bf16)  # 16.25KB
            oT = persist.tile([128, NOT, S], bf16)            # 8KB/part
            xk_sb = persist.tile([128, NIT, S], bf16)         # 32KB/part
            wq_sb = persist.tile([128, NIT, DH], bf16)        # 4KB/part
            wk_sb = persist.tile([128, NIT, DH], bf16)        # 4KB/part
            wv_sb = persist.tile([128, NIT, DH], bf16)        # 4KB/part
            wo_sb = persist.tile([128, NOT, D], bf16)         # 8KB/part
            # unnormalized-softmax denominators: rows 32*hp, cols = sq block
            denE = persist.tile([128, NSQ, 512], f32)         # 8KB/part
            denO = persist.tile([128, NSQ, 512], f32)         # 8KB/part
            bq_sb = persist.tile([128, NOT], f32)
            bk_sb = persist.tile([128, NOT], f32)

            nc.vector.memset(v_st[:, :, :, DK : DK + 1], 1.0)

            # prologue DMAs, critical path first: wq + xq chunk0 + wk + xk,
            # then wv, xv seed chunks, wo. xq streams per 512-query chunk.
            xq_tiles = {}

            def dma_xq(sqc):
                def f():
                    xq_tiles[sqc] = xqp.tile(
                        [128, NIT, 512], bf16, tag="xq", name="xq"
                    )
                    nc.sync.dma_start(
                        out=xq_tiles[sqc][:],
                        in_=xqr[:, :, 512 * sqc : 512 * (sqc + 1)],
                    )
                return f

            # critical path only: wq/wk slice 0 + xq chunk 0 + xk chunk 0.
            # Everything else is filler-emitted so its DMA doesn't steal HBM
            # bandwidth from the first projection groups.
            nc.sync.dma_start(out=wq_sb[:, :, 0:128], in_=wqr[:, :, 0:128])
            xq_tiles[0] = xqp.tile([128, NIT, 512], bf16, tag="xq", name="xq")
            nc.sync.dma_start(out=xq_tiles[0][:, 0:4, :], in_=xqr[:, 0:4, 0:512])
            nc.sync.dma_start(out=xq_tiles[0][:, 4:8, :], in_=xqr[:, 4:8, 0:512])
            nc.sync.dma_start(out=wk_sb[:, :, 0:128], in_=wkr[:, :, 0:128])
            nc.sync.dma_start(out=xk_sb[:, 0:4, 0:512], in_=xkr[:, 0:4, 0:512])
            nc.sync.dma_start(out=xk_sb[:, 4:8, 0:512], in_=xkr[:, 4:8, 0:512])
            nc.sync.dma_start(out=bq_sb[:], in_=bq[:].rearrange("(t p) -> p t", p=128))
            nc.sync.dma_start(out=bk_sb[:], in_=bk[:].rearrange("(t p) -> p t", p=128))

            xv_tiles = {}

            def dma_xk(c):
                def f():
                    nc.sync.dma_start(
                        out=xk_sb[:, :, 512 * c : 512 * (c + 1)],
                        in_=xkr[:, :, 512 * c : 512 * (c + 1)],
                    )
                return f

            def dma_xv(skt):
                def f():
                    xv_tiles[skt] = xvp.tile(
                        [128, NIT, 128], bf16, tag="xv", name="xv"
                    )
                    nc.sync.dma_start(
                        out=xv_tiles[skt][:],
                        in_=xvr[:, :, 128 * skt : 128 * (skt + 1)],
                    )
                return f

            def dma_wv():
                nc.sync.dma_start(out=wv_sb[:], in_=wvr[:])

            def dma_wo():
                nc.sync.dma_start(out=wo_sb[:], in_=wor[:])

            def dma_wslice(hp):
                def f():
                    nc.sync.dma_start(
                        out=wq_sb[:, :, 128 * hp : 128 * (hp + 1)],
                        in_=wqr[:, :, 128 * hp : 128 * (hp + 1)],
                    )
                    nc.sync.dma_start(
                        out=wk_sb[:, :, 128 * hp : 128 * (hp + 1)],
                        in_=wkr[:, :, 128 * hp : 128 * (hp + 1)],
                    )
                return f

            # ---- emission-side filler queue ----
            fillers = []
            labels = {}
            cursor = [0]

            def add(fn, label=None):
                fillers.append(fn)
                if label is not None:
                    labels[label] = len(fillers) - 1

            def pump(n):
                k = 0
                while k < n and cursor[0] < len(fillers):
                    fillers[cursor[0]]()
                    cursor[0] += 1
                    k += 1

            def pump_until(label):
                end = labels[label]
                while cursor[0] <= end:
                    fillers[cursor[0]]()
                    cursor[0] += 1

            def emit_qproj(kind, o_t, sqc):
                # one 8-matmul K-accumulation group + biased evac
                w_sb = wq_sb if kind == "q" else wk_sb
                dst = qT if kind == "q" else kt
                b_sb = bq_sb if kind == "q" else bk_sb
                ps_h = [None]

                def mk(i_t):
                    def f():
                        if i_t == 0:
                            ps_h[0] = ps_f.tile([128, 512], f32, tag="f", name="psf")
                        rhs = (
                            xq_tiles[sqc][:, i_t, :]
                            if kind == "q"
                            else xk_sb[:, i_t, 512 * sqc : 512 * (sqc + 1)]
                        )
                        nc.tensor.matmul(
                            ps_h[0][:],
                            w_sb[:, i_t, 128 * o_t : 128 * (o_t + 1)],
                            rhs,
                            start=(i_t == 0),
                            stop=(i_t == NIT - 1),
                        )
                    return f

                def evac():
                    nc.vector.tensor_scalar_add(
                        dst[:, o_t, 512 * sqc : 512 * (sqc + 1)],
                        ps_h[0][:],
                        b_sb[:, o_t : o_t + 1],
                    )
                return [mk(i) for i in range(NIT)] + [evac]

            v_done = set()

            def emit_vproj(sk_t):
                def pre():
                    # prefetch xv chunk sk_t+3
                    nxt = sk_t + 3
                    if nxt < NSK:
                        xv_tiles[nxt] = xvp.tile(
                            [128, NIT, 128], bf16, tag="xv", name="xv"
                        )
                        nc.sync.dma_start(
                            out=xv_tiles[nxt][:],
                            in_=xvr[:, :, 128 * nxt : 128 * (nxt + 1)],
                        )

                ps_h = [None]

                def mk(i_t):
                    def f():
                        if i_t == 0:
                            ps_h[0] = ps_f.tile([128, 512], f32, tag="f", name="psf")
                        nc.tensor.matmul(
                            ps_h[0][:],
                            xv_tiles[sk_t][:, i_t, :],
                            wv_sb[:, i_t, :],
                            start=(i_t == 0),
                            stop=(i_t == NIT - 1),
                        )
                    return f

                def evac():
                    nc.vector.tensor_copy(
                        v_st[:, sk_t, :, 0:DK],
                        ps_h[0][:].rearrange("p (h d) -> p h d", d=DK),
                    )
                    v_done.add(sk_t)
                return [pre] + [mk(i) for i in range(NIT)] + [evac]

            def emit_p5(sq_t, j_t):
                # partial output projection: 4-matmul accumulation, no bias
                sq_lo = 512 * sq_t
                ps_h = [None]

                def mk(o_t):
                    def f():
                        if o_t == 0:
                            ps_h[0] = ps_f.tile([128, 512], f32, tag="f", name="psf")
                        nc.tensor.matmul(
                            ps_h[0][:],
                            wo_sb[:, o_t, 128 * j_t : 128 * (j_t + 1)],
                            oT[:, o_t, sq_lo : sq_lo + 512],
                            start=(o_t == 0),
                            stop=(o_t == NOT - 1),
                        )
                    return f

                def evac():
                    y = ystg_p.tile([128, 512], f32, tag="y", name="y")
                    nc.vector.tensor_copy(y[:], ps_h[0][:])
                    nc.gpsimd.dma_start(
                        out=yT[128 * j_t : 128 * (j_t + 1), sq_lo : sq_lo + 512],
                        in_=y[:],
                    )
                return [mk(o) for o in range(NOT)] + [evac]

            # ---- static filler list (column-major block order) ----
            # Blocks run (sq0..3, h0), (sq0..3, h1), ...: K(hp) is only
            # needed before column hp, Q(hp, sq) before block (sq, hp), V
            # early (PV of column h0), P5(sq) weaves into column h3.
            vq = deque(range(NSK))

            def add_group(fns, label=None):
                for i, fn in enumerate(fns):
                    add(fn, label=label if i == len(fns) - 1 else None)

            def add_v_groups(n):
                for _ in range(n):
                    if not vq:
                        return
                    sk_t = vq.popleft()
                    add_group(emit_vproj(sk_t), label=f"V{sk_t}")

            # column h0 section: K(0, c1..3) (bulk xk DMAs filler-delayed),
            # Q(0, 1..3), and all V groups.
            add(dma_xk(1))
            add(dma_wv)
            for skt in range(3):
                add(dma_xv(skt))
            add(dma_xk(2))
            add_group(emit_qproj("k", 0, 1), label="K0c1")
            add(dma_xk(3))
            add_v_groups(2)
            add_group(emit_qproj("k", 0, 2), label="K0c2")
            add_v_groups(1)
            add(dma_xq(1))
            add_group(emit_qproj("k", 0, 3), label="K0c3")
            add_v_groups(1)
            add_group(emit_qproj("q", 0, 1), label="Q0s1")
            add_v_groups(2)
            add(dma_xq(2))
            add_group(emit_qproj("q", 0, 2), label="Q0s2")
            add_v_groups(2)
            add(dma_xq(3))
            add_group(emit_qproj("q", 0, 3), label="Q0s3")
            add_v_groups(NSK)  # remainder
            # columns h1..h3: K(hp) + Q(hp, 0..3) with xq chunk reloads
            for hp in range(1, NOT):
                add(dma_wslice(hp))
                if hp == NOT - 1:
                    add(dma_wo)
                for s in range(NSQ):
                    add(dma_xq(s))
                    add_group(emit_qproj("k", hp, s), label=f"K{hp}c{s}")
                    add_group(emit_qproj("q", hp, s), label=f"Q{hp}s{s}")

            # ---- prologue PE: Q(0, 0) + K(0, c0) emitted inline ----
            for fn in emit_qproj("q", 0, 0):
                fn()
            labels["Q0s0"] = -1
            for fn in emit_qproj("k", 0, 0):
                fn()
            labels["K0c0"] = -1

            # ---- pv backlog + in-place deferred normalization ----
            pend_pv = deque()   # (hp, sq_lo, sk_t, p_t)
            blk_po = {}         # (sq_lo, hp) -> (poE, poO)
            blocks_done = {}    # sq_lo -> list of hp
            norm_done = set()   # sq_lo values fully normalized

            p5_release = deque()  # (countdown, fns, label)

            def finish_sq(sq_lo):
                # batched in-place reciprocal of the 4+4 den rows, then
                # per-(hp,h2): bcast + normalize oT in place.
                sq_t = sq_lo // 512
                nc.vector.reciprocal(denE[:, sq_t, :], denE[:, sq_t, :])
                nc.vector.reciprocal(denO[:, sq_t, :], denO[:, sq_t, :])
                for hp in range(NOT):
                    rec2 = recp.tile([1, 2, 512], f32, tag="rec1", name="rec2")
                    nc.vector.tensor_copy(
                        rec2[0:1, 0, :], denE[32 * hp : 32 * hp + 1, sq_t, :]
                    )
                    nc.vector.tensor_copy(
                        rec2[0:1, 1, :], denO[32 * hp : 32 * hp + 1, sq_t, :]
                    )
                    bc = bcp.tile([128, 2, 512], f32, tag="bc", name="bc")
                    nc.gpsimd.partition_broadcast(bc[:], rec2[:])
                    for h2 in range(2):
                        sl = oT[64 * h2 : 64 * (h2 + 1), hp, sq_lo : sq_lo + 512]
                        nc.vector.tensor_mul(
                            sl, sl, bc[64 * h2 : 64 * (h2 + 1), h2, :]
                        )
                norm_done.add(sq_lo)
                # release P5 groups with a lag so their matmuls don't block
                # the in-order PE FIFO behind the DVE normalization chain.
                for j_t in range(NJT):
                    p5_release.append(
                        (12 + j_t, emit_p5(sq_t, j_t), f"P5_{sq_t}_{j_t}")
                    )

            def evac_block(hp, sq_lo, poE, poO):
                # evacuate unnormalized PV output + den rows; frees PSUM
                # accumulators after 4 DVE copies.
                sq_t = sq_lo // 512
                for h2, po, dt in ((0, poE, denE), (1, poO, denO)):
                    nc.vector.tensor_copy(
                        oT[64 * h2 : 64 * (h2 + 1), hp, sq_lo : sq_lo + 512],
                        po[0:DK, :],
                    )
                    nc.vector.tensor_copy(
                        dt[32 * hp : 32 * hp + 1, sq_t, :], po[DK : DK + 1, :]
                    )
                done = blocks_done.setdefault(sq_lo, [])
                done.append(hp)
                if len(done) == NOT:
                    finish_sq(sq_lo)

            steps_in_block = [0]

            def drain_pv(maxn, minlag=3, gate=True):
                k = 0
                while len(pend_pv) > minlag and k < maxn:
                    hp, sq_lo, sk_t, p_t = pend_pv[0]
                    if sk_t not in v_done:
                        return
                    if gate and sk_t == 0 and steps_in_block[0] < 2:
                        return
                    pend_pv.popleft()
                    key = (sq_lo, hp)
                    if sk_t == 0:
                        poE = psv.tile([DK + 1, 512], f32, tag="pve", name="poE")
                        poO = psv.tile([DK + 1, 512], f32, tag="pvo", name="poO")
                        blk_po[key] = (poE, poO)
                    poE, poO = blk_po[key]
                    for h2, po in ((0, poE), (1, poO)):
                        nc.tensor.matmul(
                            po[:],
                            v_st[:, sk_t, 2 * hp + h2, :],
                            p_t[:, h2, :],
                            start=(sk_t == 0),
                            stop=(sk_t == NSK - 1),
                        )
                    if sk_t == NSK - 1:
                        evac_block(hp, sq_lo, poE, poO)
                        del blk_po[key]
                    k += 1

            def force_drain(n=1):
                # hard-drain the pv backlog (pumping V fillers if the head
                # of the queue is waiting on a V projection)
                target = max(0, len(pend_pv) - n)
                while len(pend_pv) > target:
                    before = len(pend_pv)
                    drain_pv(99, minlag=0, gate=False)
                    if len(pend_pv) >= before and pend_pv:
                        if pend_pv[0][2] not in v_done:
                            pump(4)
                        else:
                            break

            def tick_release():
                n = len(p5_release)
                for _ in range(n):
                    cnt, fns, label = p5_release.popleft()
                    if cnt <= 0:
                        add_group(fns, label=label)
                    else:
                        p5_release.append((cnt - 1, fns, label))

            # ---- main attention loop (column-major over head pairs) ----
            PUMPN = {0: 4, 1: 2, 2: 2, 3: 3}
            DRAINN = {0: 3, 1: 3, 2: 3, 3: 6}
            for hp in range(NOT):
                for sq_t in range(NSQ):
                    sq_lo = 512 * sq_t
                    pump_until(f"Q{hp}s{sq_t}")
                    steps_in_block[0] = 0
                    for sk_t in range(NSK):
                        steps_in_block[0] = sk_t
                        if hp > 0 or sq_t == 0:
                            pump_until(f"K{hp}c{sk_t // 4}")
                        if len(pend_pv) >= 9:
                            force_drain(2)  # keep p-tile pool reuse safe
                        ps = ps_s.tile([128, 2, 512], f32, tag="s")
                        for h2 in range(2):
                            nc.tensor.matmul(
                                ps[:, h2, :],
                                kt[64 * h2 : 64 * (h2 + 1), hp, 128 * sk_t : 128 * (sk_t + 1)],
                                qT[64 * h2 : 64 * (h2 + 1), hp, sq_lo : sq_lo + 512],
                                start=True,
                                stop=True,
                            )
                        p_t = ppool.tile([128, 2, 512], bf16, tag="p")
                        nc.scalar.activation(
                            p_t[:], ps[:], EXP, bias=0.0, scale=0.125
                        )
                        pend_pv.append((hp, sq_lo, sk_t, p_t))
                        tick_release()
                        drain_pv(DRAINN[hp])
                        pump(PUMPN[hp])
                    if hp == NOT - 1:
                        # fire this block's evac + finish chain now so the
                        # P5 release countdown starts from a known point
                        force_drain(len(pend_pv))

            # ---- tail: drain everything, then the remaining P5 ----
            while pend_pv:
                drain_pv(99, minlag=0, gate=False)
                if pend_pv and pend_pv[0][2] not in v_done:
                    pump(4)  # make V progress
            assert len(norm_done) == NSQ, norm_done
            while p5_release:
                cnt, fns, label = p5_release.popleft()
                add_group(fns, label=label)
            pump(10**9)

    nc.compile()
    return nc


def _get_compiled():
    global _COMPILED
    if _COMPILED is None:
        _COMPILED = build()
    return _COMPILED


def make_in_maps(query, key, value, Wq, bq, Wk, bk, Wv, bv, Wo, bo):
    nbf = np.dtype("bfloat16")
    Wq = np.asarray(Wq, np.float32)
    Wk = np.asarray(Wk, np.float32)
    Wv = np.asarray(Wv, np.float32)
    Wo = np.asarray(Wo, np.float32)
    bqa = np.asarray(bq, np.float32)
    bka = np.asarray(bk, np.float32)

    xT = {}
    for b in range(B):
        xT[b] = (
            np.ascontiguousarray(np.asarray(query[b], np.float32).T).astype(nbf),
            np.ascontiguousarray(np.asarray(key[b], np.float32).T).astype(nbf),
            np.ascontiguousarray(np.asarray(value[b], np.float32).T).astype(nbf),
        )
    wslc = {}
    for hh in range(2):
        sl = slice(DH * hh, DH * (hh + 1))
        wslc[hh] = (
            np.ascontiguousarray(Wq[sl, :].T).astype(nbf),
            np.ascontiguousarray(Wk[sl, :].T).astype(nbf),
            np.ascontiguousarray(Wv[sl, :].T).astype(nbf),
            np.ascontiguousarray(Wo[:, sl].T).astype(nbf),
            np.ascontiguousarray(bqa[sl]),
            np.ascontiguousarray(bka[sl]),
        )

    in_maps = []
    for c in range(NCORES):
        b, hh = c // 2, c % 2
        xq, xk, xv = xT[b]
        wq, wk, wv, wo, bq_c, bk_c = wslc[hh]
        in_maps.append(
            {
                "xqT": xq,
                "xkT": xk,
                "xvT": xv,
                "wqT": wq,
                "wkT": wk,
                "wvT": wv,
                "woT": wo,
                "bq": bq_c,
                "bk": bk_c,
            }
        )
    return in_maps


def _gather(res, Wo, bv, bo):
    byT = (np.asarray(bo, np.float32) + np.asarray(Wo, np.float32) @ np.asarray(bv, np.float32))
    out = np.empty((B, S, D), dtype=np.float32)
    for b in range(B):
        acc = res.results[2 * b]["yT"] + res.results[2 * b + 1]["yT"]
        out[b] = acc.T + byT[None, :]
    return out


def kernel(query, key, value, mask, Wq, bq, Wk, bk, Wv, bv, Wo, bo, **_kw):
    # mask is all-ones by construction (spec fill: ones) -> no-op in softmax.
    nc = _get_compiled()
    in_maps = make_in_maps(query, key, value, Wq, bq, Wk, bk, Wv, bv, Wo, bo)
    res = run_bass_kernel_spmd(nc, in_maps, core_ids=list(range(NCORES)))
    return _gather(res, Wo, bv, bo)


def run_traced(query, key, value, mask, Wq, bq, Wk, bk, Wv, bv, Wo, bo, tmpdir=None):
    """Like kernel() but with NTFF tracing; returns (out, BassKernelResults)."""
    nc = _get_compiled()
    in_maps = make_in_maps(query, key, value, Wq, bq, Wk, bk, Wv, bv, Wo, bo)
    res = run_bass_kernel_spmd(
        nc, in_maps, core_ids=list(range(NCORES)), trace=True, tmpdir=tmpdir
    )
    return _gather(res, Wo, bv, bo), res
